# revision 22
# baseline (speedup 1.0000x reference)
"""Bipartite GNN message-passing kernel for Trainium2 (8 NeuronCores).

Strategy (v2):
  - dst is sorted -> shard queries (6250 per core); each core gets a
    contiguous edge range. No cross-core reduction needed.
  - Queries are processed in blocks of QB=120. Per block, edges are split
    into two halves by src so gather indices fit in int16 against two
    overlapping v-tables (A = obs [0,32768), B = obs [17408,50176)); the
    split point is balanced per block so each half fits 2048 edges = two
    1024-descriptor SWDGE rings exactly.
  - v = h_obs @ Wv is built once on device into 256B bf16 rows, with a
    partition-major row remap (obs o -> row (o%128)*TPP + o//128) so the
    table store is one contiguous DMA per partition.
  - hid = relu(A[dst] + pos_o @ W1o') via a one-hot mask matmul: mask is
    built q-major (2x DVE mode), PE-transposed per subtile into stack,
    PSUM->SBUF copies batched 8 subtiles at a time.
  - The distance kernel term -dist2/(2 sigma^2) + C_SHIFT is precomputed
    per edge on the host (elementwise on inputs) and loaded as `dfin`;
    softmax runs without max-subtraction (logits <= ~2, +60 shift keeps
    denominators finite in fp32).
  - Segment softmax-sum is a mask^T matmul into PSUM as before.
"""

import math
import numpy as np

N_O = 50000
N_Q = 50000
E_TOT = 1_600_000
LATENT = 128
HEADS = 4
HEAD_DIM = 32
NCORES = 8
QPC = N_Q // NCORES          # queries per core
QB = 120                     # queries per block
NBLK = math.ceil(QPC / QB)   # 53
TBL_ROWS = 32768             # rows per v-table half
B_OFF = 17408                # table B covers obs [B_OFF, B_OFF+32768)
N_PAD = TBL_ROWS * 128 // 128  # tiles-per-partition derived below
TILES = 392                  # ceil(50176/128) obs tiles (zero padded)
TPP = 256                    # table rows per partition (32768/128)
C_SHIFT = 60.0

_PROG_CACHE = {}


def _build_program(NSA, NSB, has_bv, has_b2):
    import concourse.bacc as bacc
    import concourse.bass as bass
    import concourse.mybir as mybir
    import concourse.tile as tile
    from contextlib import ExitStack

    dt = mybir.dt
    f32, bf16, i16 = dt.float32, dt.bfloat16, dt.int16
    AF = mybir.ActivationFunctionType
    OP = mybir.AluOpType

    nc = bacc.Bacc("TRN2", target_bir_lowering=False, debug=False)

    # ---- DRAM tensors (per-core inputs) ----
    hT_in = nc.dram_tensor("hT_in", [128, TILES * 128], bf16, kind="ExternalInput")
    auxA = nc.dram_tensor("auxA", [NBLK * 128, 16 * NSA], i16, kind="ExternalInput")
    auxB = nc.dram_tensor("auxB", [NBLK * 128, 16 * NSB], i16, kind="ExternalInput")
    posq_blk = nc.dram_tensor("posq_blk", [128, NBLK * 4], f32, kind="ExternalInput")
    w1qcb1 = nc.dram_tensor("w1qcb1", [4, 128], f32, kind="ExternalInput")
    w1ocf = nc.dram_tensor("w1ocf", [4, 128], f32, kind="ExternalInput")
    sel124 = nc.dram_tensor("sel124", [4, 128], f32, kind="ExternalInput")
    w2 = nc.dram_tensor("w2", [128, 4], bf16, kind="ExternalInput")
    wv = nc.dram_tensor("wv", [128, 128], bf16, kind="ExternalInput")
    b2rep = nc.dram_tensor("b2rep", [128, 4], f32, kind="ExternalInput")
    bvrep = nc.dram_tensor("bvrep", [128, 128], f32, kind="ExternalInput")
    iotaQA = nc.dram_tensor("iotaQA", [128, 124 * NSA], bf16, kind="ExternalInput")
    iotaQB = nc.dram_tensor("iotaQB", [128, 124 * NSB], bf16, kind="ExternalInput")
    idf32 = nc.dram_tensor("idf32", [128, 128], f32, kind="ExternalInput")
    idbf = nc.dram_tensor("idbf", [128, 128], bf16, kind="ExternalInput")

    GA = nc.dram_tensor("GA", [TBL_ROWS, LATENT], bf16)
    GB = nc.dram_tensor("GB", [TBL_ROWS, LATENT], bf16)
    out_d = nc.dram_tensor("out", [NBLK * QB, 128], f32, kind="ExternalOutput")

    with tile.TileContext(nc) as tc, ExitStack() as ctx:
        cpool = ctx.enter_context(tc.tile_pool(name="consts", bufs=1))
        aw1_sb = cpool.tile([128, NBLK * 128], bf16, tag="aw1")

        idb_sb = cpool.tile([128, 128], bf16, tag="idb")
        nc.sync.dma_start(idb_sb[:], idbf[:])
        idf_sb = cpool.tile([128, 128], f32, tag="idf")
        nc.sync.dma_start(idf_sb[:], idf32[:])
        w1qc_sb = cpool.tile([4, 128], f32, tag="w1qc")
        nc.sync.dma_start(w1qc_sb[:], w1qcb1[:])
        w1ocf_sb = cpool.tile([4, 128], f32, tag="w1ocf")
        nc.sync.dma_start(w1ocf_sb[:], w1ocf[:])
        sel_sb = cpool.tile([4, 128], f32, tag="sel")
        nc.sync.dma_start(sel_sb[:], sel124[:])
        w2_sb = cpool.tile([128, 4], bf16, tag="w2")
        nc.sync.dma_start(w2_sb[:], w2[:])
        wv_sb = cpool.tile([128, 128], bf16, tag="wv")
        nc.sync.dma_start(wv_sb[:], wv[:])
        ioA_sb = cpool.tile([128, 124 * NSA], bf16, tag="ioA")
        nc.sync.dma_start(ioA_sb[:], iotaQA[:])
        ioB_sb = cpool.tile([128, 124 * NSB], bf16, tag="ioB")
        nc.sync.dma_start(ioB_sb[:], iotaQB[:])
        pq_sb = cpool.tile([128, NBLK * 4], f32, tag="pq")
        nc.sync.dma_start(pq_sb[:], posq_blk[:])
        if has_b2:
            b2_sb = cpool.tile([128, 4], f32, tag="b2")
            nc.sync.dma_start(b2_sb[:], b2rep[:])
        if has_bv:
            bv_sb = cpool.tile([128, 128], f32, tag="bv")
            nc.sync.dma_start(bv_sb[:], bvrep[:])

        # ---------- prologue A: v table (v = hT.T @ Wv, bf16, 256B rows) ----
        # hT uploaded pre-transposed [latent, obs] bf16; Wv uploaded with
        # columns permuted (w,h)-major so vse's attn broadcast is 2x-capable.
        # Table row remap (host side for gather idx): o -> (o%128)*TPP + o//128
        # so the store per partition is contiguous.
        CH = 98  # tiles per load chunk (392 = 4*98)
        GAv_st = GA[:].rearrange("(p t) d -> p (t d)", p=128)
        GBv_st = GB[:].rearrange("(p t) d -> p (t d)", p=128)
        BT = B_OFF // 128  # 136
        # store parts emitted as soon as their tile range is computed:
        # chunk c covers tiles [49c, 49c+49)
        store_sched = {
            0: [(GAv_st, 0, 0, 64)],          # (dst, dst_t0, src_t0, ntiles)
            1: [(GAv_st, 64, 64, 64), (GAv_st, 128, 128, 64)],
            2: [(GAv_st, 192, 192, 64), (GBv_st, 0, BT, 64),
                (GBv_st, 64, BT + 64, 64)],
            3: [(GBv_st, 128, BT + 128, 64), (GBv_st, 192, BT + 192, 64)],
        }
        with tc.tile_pool(name="vt", bufs=1) as vt, \
             tc.tile_pool(name="ld", bufs=2) as lp, \
             tc.tile_pool(name="ht", bufs=2) as hp, \
             tc.tile_pool(name="ps_t", bufs=2, space="PSUM") as pst, \
             tc.tile_pool(name="ps_v", bufs=2, space="PSUM") as psv:
            vtab = vt.tile([128, TILES * 128], bf16, tag="vtab")
            for c in range(4):
                ld = lp.tile([128, CH * 128], bf16, tag="ld")
                nc.sync.dma_start(ld[:], hT_in[:, c * CH * 128:(c + 1) * CH * 128])
                for g0 in range(0, CH, 4):
                    gn = min(4, CH - g0)
                    vp = psv.tile([128, 512], f32, tag="vp", space="PSUM")
                    for j in range(gn):
                        nc.tensor.matmul(
                            out=vp[:, j * 128:(j + 1) * 128],
                            lhsT=ld[:, (g0 + j) * 128:(g0 + j + 1) * 128],
                            rhs=wv_sb[:], start=True, stop=True)
                    dst_lo = (c * CH + g0) * 128
                    if has_bv:
                        vs = hp.tile([128, 512], f32, tag="vs")
                        nc.vector.tensor_tensor(
                            out=vs[:, 0:gn * 128].rearrange("p (j d) -> p j d", d=128),
                            in0=vp[:, 0:gn * 128].rearrange("p (j d) -> p j d", d=128),
                            in1=bv_sb[:].unsqueeze(1).broadcast_to([128, gn, 128]),
                            op=OP.add)
                        nc.scalar.activation(
                            out=vtab[:, dst_lo:dst_lo + gn * 128],
                            in_=vs[:, 0:gn * 128], func=AF.Copy, bias=0.0, scale=1.0)
                    elif (g0 // 4) % 4 == 0:
                        nc.scalar.activation(
                            out=vtab[:, dst_lo:dst_lo + gn * 128],
                            in_=vp[:, 0:gn * 128], func=AF.Copy, bias=0.0, scale=1.0)
                    else:
                        nc.vector.tensor_copy(
                            out=vtab[:, dst_lo:dst_lo + gn * 128],
                            in_=vp[:, 0:gn * 128])
                for dstv, d0, s0, nt in store_sched.get(c, ()):
                    nc.sync.dma_start(
                        dstv[:, d0 * 128:(d0 + nt) * 128],
                        vtab[:, s0 * 128:(s0 + nt) * 128])

            # ---------- prologue B: per-block A-table (aw1) ----------
            for b in range(NBLK):
                pq4 = hp.tile([128, 4], f32, tag="pq4")
                nc.vector.tensor_copy(out=pq4[:], in_=pq_sb[:, b * 4:b * 4 + 4])
                tps = pst.tile([128, 128], f32, tag="tps", space="PSUM")
                nc.tensor.transpose(out=tps[0:4, :], in_=pq4[:], identity=idf_sb[:])
                pqT = hp.tile([4, 128], f32, tag="pqT")
                nc.scalar.copy(out=pqT[:], in_=tps[0:4, :])
                aps = psv.tile([128, 128], f32, tag="aps", space="PSUM")
                nc.tensor.matmul(out=aps[:], lhsT=pqT[:], rhs=w1qc_sb[:],
                                 start=True, stop=False)
                nc.tensor.matmul(out=aps[:], lhsT=sel_sb[:], rhs=w1ocf_sb[:],
                                 start=False, stop=True)
                nc.vector.tensor_copy(out=aw1_sb[:, b * 128:(b + 1) * 128],
                                      in_=aps[:])

        # ---------- main edge loop ----------
        fpool = ctx.enter_context(tc.tile_pool(name="fmain", bufs=5))
        bpool = ctx.enter_context(tc.tile_pool(name="bmain", bufs=4))
        spool = ctx.enter_context(tc.tile_pool(name="small", bufs=4))
        tpool = ctx.enter_context(tc.tile_pool(name="psT", bufs=2, space="PSUM"))
        hpool = ctx.enter_context(tc.tile_pool(name="psH", bufs=2, space="PSUM"))
        qpool = ctx.enter_context(tc.tile_pool(name="psQ", bufs=1, space="PSUM"))
        opool = ctx.enter_context(tc.tile_pool(name="psO", bufs=1, space="PSUM"))

        GAv = GA[:]
        GBv = GB[:]
        G = 8  # subtiles per relu/copy batch

        # Software pipeline over half-slots: frontend(i) overlaps
        # backend(i-2); the pout scatter runs one slot later still so its
        # vse operand is a full slot old. This keeps every engine's
        # in-order queue free of head-of-line blocking.
        halves = [(b, h) for b in range(NBLK) for h in range(2)]
        NH = len(halves)
        state = {}
        pout_cur = [None]

        def frontend_a(i):
            b, half = halves[i]
            NS = NSA if half == 0 else NSB
            gsrc = GAv if half == 0 else GBv
            aux = auxA if half == 0 else auxB
            io_sb = ioA_sb if half == 0 else ioB_sb
            auxt = fpool.tile([128, 16 * NS], i16, tag="auxt")
            nc.sync.dma_start(auxt[:], aux[b * 128:(b + 1) * 128, :])
            idxs = auxt[:, 2 * NS:10 * NS]
            drt = auxt[:, 10 * NS:11 * NS].bitcast(bf16)
            posE = auxt[:, 11 * NS:15 * NS].bitcast(bf16)
            gt = fpool.tile([128, NS * 128], bf16, tag="gt")
            gv = gt[:].rearrange("p (n k) -> p n k", k=128)
            for c0 in range(0, NS, 8):
                c1 = min(c0 + 8, NS)
                n_c = (c1 - c0) * 128
                nc.gpsimd.dma_gather(
                    out_ap=gv[:, c0:c1, :],
                    in_ap=gsrc,
                    idxs_ap=idxs[:, c0 * 8:c0 * 8 + n_c // 16],
                    num_idxs=n_c,
                    num_idxs_reg=n_c,
                    elem_size=128,
                )
            # mask, q-major columns: col (q, n) = q*NS + n
            mext = fpool.tile([128, NS * 128], bf16, tag="mext")
            nc.vector.tensor_tensor(
                out=mext[:, 0:124 * NS].rearrange("p (q n) -> p q n", n=NS),
                in0=io_sb[:].rearrange("p (q n) -> p q n", n=NS),
                in1=drt.unsqueeze(1).broadcast_to([128, 124, NS]),
                op=OP.is_equal)
            nc.gpsimd.tensor_copy(out=mext[:, 124 * NS:128 * NS], in_=posE)
            state[i] = dict(NS=NS, b=b, half=half, auxt=auxt, gt=gt,
                            mext=mext)

        def frontend_b(i):
            st = state[i]
            NS = st["NS"]
            mv = st["mext"][:].rearrange("p (q n) -> p q n", n=NS)
            stack = fpool.tile([128, NS * 128], bf16, tag="stack")
            for j in range(NS):
                if j % G == 0:
                    tp = tpool.tile([128, G * 128], bf16, tag="tp",
                                    space="PSUM")
                nc.tensor.transpose(
                    out=tp[:, (j % G) * 128:((j % G) + 1) * 128],
                    in_=mv[:, :, j],
                    identity=idb_sb[:])
                if j % G == G - 1 or j == NS - 1:
                    lo = (j // G) * G * 128
                    w = ((j % G) + 1) * 128
                    # split PSUM->SBUF copies between scalar and vector
                    if (j // G) % 2 == 0:
                        nc.scalar.copy(out=stack[:, lo:lo + w],
                                       in_=tp[:, 0:w])
                    else:
                        nc.vector.tensor_copy(out=stack[:, lo:lo + w],
                                              in_=tp[:, 0:w])
            st["stack"] = stack

        def backend_mid(i):
            st = state[i]
            NS, b = st["NS"], st["b"]
            stack = st["stack"]
            hid = bpool.tile([128, NS * 128], bf16, tag="hid")
            qdv = qpool.tile([128, NS * 4], f32, tag="qdv", space="PSUM")
            for j in range(NS):
                if j % G == 0:
                    ph = hpool.tile([128, G * 128], f32, tag="ph",
                                    space="PSUM")
                nc.tensor.matmul(
                    out=ph[:, (j % G) * 128:((j % G) + 1) * 128],
                    lhsT=aw1_sb[:, b * 128:(b + 1) * 128],
                    rhs=stack[:, j * 128:(j + 1) * 128],
                    start=True, stop=True)
                if j % G == G - 1 or j == NS - 1:
                    lo = (j // G) * G * 128
                    w = ((j % G) + 1) * 128
                    nc.scalar.activation(
                        out=hid[:, lo:lo + w], in_=ph[:, 0:w],
                        func=AF.Relu, bias=0.0, scale=1.0)
            for j in range(NS):
                nc.tensor.matmul(
                    out=qdv[:, j * 4:(j + 1) * 4],
                    lhsT=hid[:, j * 128:(j + 1) * 128],
                    rhs=w2_sb[:],
                    start=True, stop=True)
            st["qdv"] = qdv

        def backend_tail(i):
            st = state[i]
            NS = st["NS"]
            dfin = st["auxt"][:, 0:2 * NS].bitcast(f32)
            qdv = st["qdv"]
            gv = st["gt"][:].rearrange("p (n k) -> p n k", k=128)
            lst = spool.tile([128, NS * 4], f32, tag="lst")
            nc.vector.tensor_tensor(
                out=lst[:].rearrange("p (n h) -> p n h", h=4),
                in0=qdv[:].rearrange("p (n h) -> p n h", h=4),
                in1=dfin.unsqueeze(2).broadcast_to([128, NS, 4]),
                op=OP.add)
            if has_b2:
                nc.gpsimd.tensor_tensor(
                    out=lst[:].rearrange("p (n h) -> p n h", h=4),
                    in0=lst[:].rearrange("p (n h) -> p n h", h=4),
                    in1=b2_sb[:].unsqueeze(1).broadcast_to([128, NS, 4]),
                    op=OP.add)
            ex = spool.tile([128, NS * 4], bf16, tag="ex")
            nc.scalar.activation(out=ex[:], in_=lst[:], func=AF.Exp,
                                 bias=0.0, scale=1.0)
            # v is stored (w,h)-major, so the attn broadcast sits on the
            # middle dim and the op is 2x-eligible (contiguous last dim).
            vse = bpool.tile([128, NS * 132], bf16, tag="vse")
            vsev = vse[:].rearrange("p (n k) -> p n k", k=132)
            nc.vector.tensor_tensor(
                out=vsev[:, :, 0:128].rearrange("p n (w h) -> p n w h", h=4),
                in0=gv.rearrange("p n (w h) -> p n w h", h=4),
                in1=ex[:].rearrange("p (n h) -> p n h", h=4).unsqueeze(2)
                    .broadcast_to([128, NS, 32, 4]),
                op=OP.mult)
            nc.gpsimd.tensor_copy(
                out=vsev[:, :, 128:132],
                in_=ex[:].rearrange("p (n h) -> p n h", h=4))
            st["vse"] = vse

        def backend_pout(i):
            st = state[i]
            NS, half = st["NS"], st["half"]
            mv = st["mext"][:].rearrange("p (q n) -> p q n", n=NS)
            vse = st["vse"]
            if half == 0:
                pout = opool.tile([128, 132], f32, tag="pout", space="PSUM")
                pout_cur[0] = pout
            pout = pout_cur[0]
            for j in range(NS):
                nc.tensor.matmul(
                    out=pout[0:124, :],
                    lhsT=mv[:, 0:124, j],
                    rhs=vse[:, j * 132:(j + 1) * 132],
                    start=(half == 0 and j == 0),
                    stop=(half == 1 and j == NS - 1))

        def block_finish(i):
            st = state[i]
            b = st["b"]
            pout = pout_cur[0]
            den = spool.tile([128, 4], f32, tag="den")
            nc.scalar.activation(out=den[0:124, :], in_=pout[0:124, 128:132],
                                 func=AF.Copy, bias=1e-30, scale=1.0)
            rec = spool.tile([128, 4], f32, tag="rec")
            nc.vector.reciprocal(out=rec[0:124, :], in_=den[0:124, :])
            onorm = spool.tile([128, 128], f32, tag="onorm")
            nc.vector.tensor_tensor(
                out=onorm[0:124, :].rearrange("p (w h) -> p w h", h=4),
                in0=pout[0:124, 0:128].rearrange("p (w h) -> p w h", h=4),
                in1=rec[0:124, :].unsqueeze(1).broadcast_to([124, 32, 4]),
                op=OP.mult)
            nc.sync.dma_start(out_d[b * QB:(b + 1) * QB, :], onorm[0:QB, :])

        for i in range(NH + 4):
            if i < NH:
                frontend_a(i)
            if 2 <= i < NH + 2:
                backend_mid(i - 2)
            if 3 <= i < NH + 3:
                backend_tail(i - 3)
            if i >= 4:
                backend_pout(i - 4)
                if halves[i - 4][1] == 1:
                    block_finish(i - 4)
            if i < NH:
                frontend_b(i)
            if i >= 4:
                state.pop(i - 4)

    nc.compile()
    return nc


def _host_prep(h_obs, pos_obs, pos_query, src, dst, W1, b1, W2, b2, Wv, bv,
               log_sigma):
    import ml_dtypes
    bf = ml_dtypes.bfloat16

    src = np.asarray(src).astype(np.int64)
    dst = np.asarray(dst).astype(np.int64)
    h_obs = np.asarray(h_obs, dtype=np.float32)
    pos_obs = np.asarray(pos_obs, dtype=np.float32)
    pos_query = np.asarray(pos_query, dtype=np.float32)
    W1 = np.asarray(W1, dtype=np.float32)
    W2 = np.asarray(W2, dtype=np.float32)
    Wv = np.asarray(Wv, dtype=np.float32)
    b1 = np.asarray(b1, dtype=np.float32)
    b2 = np.asarray(b2, dtype=np.float32)
    bv = np.asarray(bv, dtype=np.float32)
    sigma = np.exp(np.float64(np.float32(log_sigma))) + np.float64(np.float32(1e-6))
    inv_sig2 = float(1.0 / (sigma * sigma))

    has_bv = bool(np.any(bv))
    has_b2 = bool(np.any(b2))

    # per-core edge partition; per (core, block) balanced A/B half split
    edge_bounds = np.searchsorted(dst, np.arange(NCORES + 1) * QPC)
    # remap obs id -> table row
    def rowmap(o):
        return ((o % 128) * TPP + o // 128).astype(np.int16)

    core_lists = []  # [core][block][half] -> (rows_i16, drel_f, pos_f, dfin_f)
    max_nsa = 1
    max_nsb = 1
    for c in range(NCORES):
        e0, e1 = edge_bounds[c], edge_bounds[c + 1]
        dl = dst[e0:e1] - c * QPC
        sl = src[e0:e1]
        blocks = []
        blk_bounds = np.searchsorted(dl, np.arange(NBLK + 1) * QB)
        for b in range(NBLK):
            be0, be1 = blk_bounds[b], blk_bounds[b + 1]
            bsrc = sl[be0:be1]
            bdr = (dl[be0:be1] - b * QB).astype(np.float32)
            # balanced split: A must take src < B_OFF, B must take
            # src >= 32768; middle zone fills A up to 2048.
            mustA = bsrc < B_OFF
            mustB = bsrc >= TBL_ROWS
            mid = ~mustA & ~mustB
            roomA = max(0, min(2048, int(len(bsrc)) - int(mustB.sum())) -
                        int(mustA.sum()))
            mid_idx = np.nonzero(mid)[0]
            inA = mustA.copy()
            if roomA > 0 and len(mid_idx) > 0:
                inA[mid_idx[:roomA]] = True
            halves = []
            for hm, off in ((inA, 0), (~inA, B_OFF)):
                o_loc = bsrc[hm] - off
                rows = rowmap(o_loc)
                dr = bdr[hm]
                po = pos_obs[bsrc[hm]]
                pq = pos_query[dst[e0:e1][be0:be1][hm]]
                rel = pq - po
                d2 = np.sum(rel.astype(np.float64) ** 2, axis=1)
                dfe = (C_SHIFT - d2 * inv_sig2 * 0.5).astype(np.float32)
                halves.append((rows, dr, po, dfe))
            max_nsa = max(max_nsa, math.ceil(len(halves[0][0]) / 128))
            max_nsb = max(max_nsb, math.ceil(len(halves[1][0]) / 128))
            blocks.append(halves)
        core_lists.append(blocks)
    NSA, NSB = max_nsa, max_nsb

    # constants
    ident = np.eye(128, dtype=np.float32)
    w1qcb1 = np.concatenate([W1[0:3] + W1[3:6], b1[None, :]], 0).astype(np.float32)
    w1oc = np.zeros((4, 128), np.float32)
    w1oc[0:3] = W1[6:9] - W1[0:3]

    # (w,h)-major permutation of the value dimension
    Wv_p = Wv.reshape(LATENT, HEADS, HEAD_DIM).transpose(0, 2, 1) \
        .reshape(LATENT, 128)
    bv_p = bv.reshape(HEADS, HEAD_DIM).T.reshape(128)

    hpad = np.zeros((TILES * 128, LATENT), np.float32)
    hpad[:N_O] = h_obs
    hT = np.ascontiguousarray(hpad.astype(bf).T)  # [128, 50176] bf16

    def iotaQ(ns):
        q = np.arange(124, dtype=np.float32)
        m = np.repeat(q, ns)[None, :]
        return np.broadcast_to(m, (128, 124 * ns)).astype(bf)

    def pack_aux(halves_list, NS):
        # aux row layout per partition (i16 cols):
        # [0:2NS) dfin f32, [2NS:10NS) idx i16, [10NS:11NS) drt bf16,
        # [11NS:15NS) posE bf16 (3 coords + zeros), [15NS:16NS) pad to 512B
        NSP = NS * 128
        aux = np.zeros((NBLK, 128, 16 * NS), np.int16)
        for b in range(NBLK):
            rows, dr, po, dfe = halves_list[b]
            n = len(rows)
            ip = np.zeros(NSP, np.int16)
            ip[:n] = rows
            w = ip.reshape(NSP // 16, 16).T  # [16, NS*8]
            aux[b, :, 2 * NS:10 * NS] = np.tile(w, (8, 1))
            dp = np.full(NSP, -1.0, np.float32)
            dp[:n] = dr
            aux[b, :, 10 * NS:11 * NS] = (
                dp.reshape(NS, 128).T.astype(bf).view(np.int16))
            pp = np.zeros((NSP, 4), np.float32)
            pp[:n, 0:3] = po
            # posE layout [128, (i, n)]: col i*NS + n
            pe = pp.reshape(NS, 128, 4).transpose(1, 2, 0)  # [128, 4, NS]
            aux[b, :, 11 * NS:15 * NS] = (
                pe.reshape(128, 4 * NS).astype(bf).view(np.int16))
            df = np.full(NSP, -40.0, np.float32)
            df[:n] = dfe
            aux[b, :, 0:2 * NS] = (
                df.reshape(NS, 128).T.copy().view(np.int16).reshape(128, 2 * NS))
        return aux.reshape(NBLK * 128, 16 * NS)

    in_maps = []
    for c in range(NCORES):
        auxA = pack_aux([core_lists[c][b][0] for b in range(NBLK)], NSA)
        auxB = pack_aux([core_lists[c][b][1] for b in range(NBLK)], NSB)
        pqb = np.zeros((128, NBLK * 4), np.float32)
        qs = pos_query[c * QPC:(c + 1) * QPC]
        for b in range(NBLK):
            lo, hi = b * QB, min((b + 1) * QB, QPC)
            pqb[:hi - lo, b * 4:b * 4 + 3] = qs[lo:hi]
            pqb[:hi - lo, b * 4 + 3] = 1.0
        in_maps.append({
            "hT_in": hT,
            "auxA": auxA, "auxB": auxB,
            "posq_blk": pqb,
            "w1qcb1": w1qcb1,
            "w1ocf": w1oc,
            "sel124": ident[124:128],
            "w2": W2.astype(bf),
            "wv": Wv_p.astype(bf),
            "b2rep": np.broadcast_to(b2, (128, 4)).copy().astype(np.float32),
            "bvrep": np.broadcast_to(bv_p, (128, 128)).copy().astype(np.float32),
            "iotaQA": iotaQ(NSA),
            "iotaQB": iotaQ(NSB),
            "idf32": ident,
            "idbf": ident.astype(bf),
        })
    return NSA, NSB, has_bv, has_b2, in_maps


def kernel(h_obs, pos_obs, pos_query, src, dst, W1, b1, W2, b2, Wv, bv,
           log_sigma, **_unused):
    import sys
    for p in ("/opt/trn_rl_repo", "/root/.axon_site/_ro/trn_rl_repo"):
        if p not in sys.path:
            sys.path.append(p)
    from concourse.bass_utils import run_bass_kernel_spmd

    NSA, NSB, has_bv, has_b2, in_maps = _host_prep(
        h_obs, pos_obs, pos_query, src, dst, W1, b1, W2, b2, Wv, bv, log_sigma)

    key = (NSA, NSB, has_bv, has_b2)
    if key not in _PROG_CACHE:
        _PROG_CACHE[key] = _build_program(NSA, NSB, has_bv, has_b2)
    nc = _PROG_CACHE[key]

    res = run_bass_kernel_spmd(nc, in_maps, core_ids=list(range(NCORES)))
    outs = [np.asarray(r["out"])[:QPC] for r in res.results]
    full = np.concatenate(outs, axis=0).astype(np.float32)
    # undo the (w,h)-major value permutation
    return np.ascontiguousarray(
        full.reshape(N_Q, HEAD_DIM, HEADS).transpose(0, 2, 1).reshape(N_Q, 128))


if __name__ == "__main__":
    pass
